# revision 1
# baseline (speedup 1.0000x reference)
"""Trainium2 Bass kernel: multiscale K-Planes lookup + MLP (v2).

Design (vs the dma_gather baseline):
  - All 12 (scale,plane) bilinear lookups per point run through the
    GPSIMD ap_gather (SBUF->SBUF), whose per-index cost is ~28ns per
    Q7 core but parallel across the 8 cores.  Grouping: partition
    group g = (point_parity h, scale s); slot = (sigma = n//2, plane).
    All 8 Q7 cores process concurrently => ~42 ns/pt total.
  - Tables are fp16 "delta rows" (v00, dy, dx, dxy per channel) living
    in SBUF, reloaded per y-bucket (YB=8 rows) via sequential DMA.
  - Host precomputes idx lists and fp16 interp weights; on device the
    weights are partition-broadcast with one K=8 one-hot matmul.
  - Interp in feature-major layout [128=(h,s,c), slots]; plane product
    = stride-3 slot ops; the MLP consumes feats [64=(s,c), sigma]
    directly (no input transposes). Output transposed back via PE.
"""

import math
import numpy as np
from contextlib import ExitStack

import concourse.bass as bass
import concourse.bacc as bacc
import concourse.mybir as mybir
import concourse.tile as tile
from concourse import library_config

FP = mybir.dt.float32
F16 = mybir.dt.float16
I16 = mybir.dt.int16

H = 150
WS = (64, 128, 256, 512)
NP = 3
NS = 4
NCORES = 8
YB = 8                          # y-bucket height (exact, host-computed iy0)
NBKT = (H - 1 + YB - 1) // YB   # 19
N = 1024                        # points per tile
NSLOT = (N // 2) * NP           # 1536 slots per gather group
ALU = mybir.AluOpType


def bucket_rows(b):
    lo = b * YB
    hi = min(H - 1, lo + YB)
    return lo, hi - lo            # window [lo, lo+win) covers iy0c


def cells_per_row(s):
    return NP * (WS[s] - 1)


def win_cells(s, b):
    _, win = bucket_rows(b)
    return win * cells_per_row(s)


MAX_CELLS = max(win_cells(s, 0) for s in range(NS))   # s3: 8*3*511 = 12264


# ---------------------------------------------------------------------------
# device program
# ---------------------------------------------------------------------------

def build_program(tiles_per_bucket, num_devices=1):
    nc = bacc.Bacc("TRN2", target_bir_lowering=False, debug=False,
                   enable_asserts=False, num_devices=num_devices)

    nb = int(sum(tiles_per_bucket))
    L = nb * N

    # full per-scale tables [16ch, (H-1)*3*(W-1)*4] fp16
    tabs_d = [nc.dram_tensor(f"tab{s}", [16, (H - 1) * cells_per_row(s) * 4],
                             F16, kind="ExternalInput").ap()
              for s in range(NS)]
    idx_d = nc.dram_tensor("idx", [nb, 128, NSLOT // 16], I16,
                           kind="ExternalInput").ap()
    wx_d = nc.dram_tensor("wx", [nb, 8, NSLOT], F16, kind="ExternalInput").ap()
    wy_d = nc.dram_tensor("wy", [nb, 8, N // 2], F16, kind="ExternalInput").ap()
    sel8_d = nc.dram_tensor("sel8", [8, 128], F16, kind="ExternalInput").ap()
    w0_d = nc.dram_tensor("w0arr", [128, 128], F16, kind="ExternalInput").ap()
    w1_d = nc.dram_tensor("w1t", [128, 128], F16, kind="ExternalInput").ap()
    w2_d = nc.dram_tensor("w2t", [128, 64], F16, kind="ExternalInput").ap()
    b0_d = nc.dram_tensor("b0c", [128, 1], FP, kind="ExternalInput").ap()
    b1_d = nc.dram_tensor("b1c", [128, 1], FP, kind="ExternalInput").ap()
    b2_d = nc.dram_tensor("b2r", [128, 64], FP, kind="ExternalInput").ap()
    out_d = nc.dram_tensor("out", [L, 64], FP, kind="ExternalOutput").ap()

    from concourse.masks import make_identity

    with tile.TileContext(nc) as tc:
        with ExitStack() as ctx:
            cpool = ctx.enter_context(tc.tile_pool(name="cpool", bufs=1))
            tpool = ctx.enter_context(tc.tile_pool(name="tpool", bufs=1))
            ipool = ctx.enter_context(tc.tile_pool(name="ipool", bufs=2))
            gpool = ctx.enter_context(tc.tile_pool(name="gpool", bufs=2))
            vpool = ctx.enter_context(tc.tile_pool(name="vpool", bufs=2))
            mpool = ctx.enter_context(tc.tile_pool(name="mpool", bufs=2))
            opool = ctx.enter_context(tc.tile_pool(name="opool", bufs=2))
            qpool = ctx.enter_context(tc.tile_pool(name="qpool", bufs=1,
                                                   space="PSUM"))

            nc.gpsimd.load_library(library_config.ap_gather)

            ident = cpool.tile([128, 128], FP)
            make_identity(nc, ident)
            sel8 = cpool.tile([8, 128], F16)
            nc.sync.dma_start(sel8, sel8_d)
            w0arr = cpool.tile([128, 128], F16)
            nc.sync.dma_start(w0arr, w0_d)
            w1t = cpool.tile([128, 128], F16)
            nc.sync.dma_start(w1t, w1_d)
            w2t = cpool.tile([128, 64], F16)
            nc.sync.dma_start(w2t, w2_d)
            b0 = cpool.tile([128, 1], FP)
            nc.sync.dma_start(b0, b0_d)
            b1 = cpool.tile([128, 1], FP)
            nc.sync.dma_start(b1, b1_d)
            b2r = cpool.tile([128, 64], FP)
            nc.sync.dma_start(b2r, b2_d)

            # one resident table tile; per bucket, reload the window for
            # each scale into partitions (4h+s)*16 .. +16 (h = 0 and 1).
            tabt = tpool.tile([128, MAX_CELLS * 4], F16)

            blk = 0
            for b in range(NBKT):
                lo, win = bucket_rows(b)
                for s in range(NS):
                    base = lo * cells_per_row(s) * 4
                    nbytes = win * cells_per_row(s) * 4
                    for h in range(2):
                        g = 4 * h + s
                        nc.sync.dma_start(
                            tabt[16 * g:16 * g + 16, 0:nbytes],
                            tabs_d[s][:, base:base + nbytes])

                for _ in range(int(tiles_per_bucket[b])):
                    # ---- inputs ----
                    idxt = ipool.tile([128, NSLOT // 16], I16, tag="idx")
                    nc.sync.dma_start(idxt, idx_d[blk])
                    wxr = ipool.tile([8, NSLOT], F16, tag="wxr")
                    nc.sync.dma_start(wxr, wx_d[blk])
                    wyr = ipool.tile([8, N // 2], F16, tag="wyr")
                    nc.sync.dma_start(wyr, wy_d[blk])

                    # ---- weight broadcast (K=8 one-hot matmul) ----
                    wx = vpool.tile([128, NSLOT], F16, tag="wx")
                    for c in range(NSLOT // 512):
                        pwx = qpool.tile([128, 512], FP, space="PSUM",
                                         tag="pwx")
                        nc.tensor.matmul(out=pwx, lhsT=sel8,
                                         rhs=wxr[:, c * 512:(c + 1) * 512],
                                         start=True, stop=True)
                        nc.scalar.activation(
                            wx[:, c * 512:(c + 1) * 512], pwx,
                            mybir.ActivationFunctionType.Copy)
                    wy = vpool.tile([128, N // 2], F16, tag="wy")
                    pwy = qpool.tile([128, N // 2], FP, space="PSUM",
                                     tag="pwy")
                    nc.tensor.matmul(out=pwy, lhsT=sel8, rhs=wyr,
                                     start=True, stop=True)
                    nc.scalar.activation(wy, pwy,
                                         mybir.ActivationFunctionType.Copy)

                    # ---- gather ----
                    g = gpool.tile([128, NSLOT * 4], F16, tag="g")
                    nc.gpsimd.ap_gather(
                        out_ap=g.rearrange("p (i d) -> p i d", d=4),
                        in_ap=tabt.rearrange("p (c d) -> p c d", d=4),
                        idxs_ap=idxt,
                        channels=128, num_elems=MAX_CELLS, d=4,
                        num_idxs=NSLOT)

                    # ---- interp: val = (v00 + wx*dx) + wy*(dy + wx*dxy) --
                    gv = g.rearrange("p (i d) -> p i d", d=4)
                    m = vpool.tile([128, NSLOT], F16, tag="m")
                    A = vpool.tile([128, NSLOT], F16, tag="A")
                    B = vpool.tile([128, NSLOT], F16, tag="B")
                    val = vpool.tile([128, NSLOT], F16, tag="val")
                    nc.vector.tensor_tensor(out=m, in0=gv[:, :, 2], in1=wx,
                                            op=ALU.mult)
                    nc.vector.tensor_tensor(out=A, in0=gv[:, :, 0], in1=m,
                                            op=ALU.add)
                    nc.vector.tensor_tensor(out=m, in0=gv[:, :, 3], in1=wx,
                                            op=ALU.mult)
                    nc.vector.tensor_tensor(out=B, in0=gv[:, :, 1], in1=m,
                                            op=ALU.add)
                    wyb = wy.unsqueeze(-1).to_broadcast([128, N // 2, NP])
                    nc.vector.tensor_tensor(
                        out=B.rearrange("p (q r) -> p q r", r=NP),
                        in0=B.rearrange("p (q r) -> p q r", r=NP),
                        in1=wyb, op=ALU.mult)
                    nc.vector.tensor_tensor(out=val, in0=A, in1=B, op=ALU.add)

                    # ---- plane product (slot stride 3) ----
                    v3 = val.rearrange("p (q r) -> p q r", r=NP)
                    pp = vpool.tile([128, N // 2], F16, tag="pp")
                    feats = vpool.tile([128, N // 2], F16, tag="feats")
                    nc.vector.tensor_tensor(out=pp, in0=v3[:, :, 0],
                                            in1=v3[:, :, 1], op=ALU.mult)
                    nc.vector.tensor_tensor(out=feats, in0=pp,
                                            in1=v3[:, :, 2], op=ALU.mult)

                    # ---- MLP per parity half ----
                    outt = opool.tile([128, (N // 128) * 64], FP, tag="outt")
                    for h in range(2):
                        rhs = feats[64 * h:64 * h + 64, :]
                        p0 = qpool.tile([128, N // 2], FP, space="PSUM",
                                        tag="p0")
                        nc.tensor.matmul(out=p0,
                                         lhsT=w0arr[64 * h:64 * h + 64, :],
                                         rhs=rhs, start=True, stop=True)
                        h0 = mpool.tile([128, N // 2], F16, tag="h0")
                        nc.scalar.activation(h0, p0,
                                             mybir.ActivationFunctionType.Relu,
                                             bias=b0[:, 0:1])
                        p1 = qpool.tile([128, N // 2], FP, space="PSUM",
                                        tag="p1")
                        nc.tensor.matmul(out=p1, lhsT=w1t, rhs=h0,
                                         start=True, stop=True)
                        h1 = mpool.tile([128, N // 2], F16, tag="h1")
                        nc.scalar.activation(h1, p1,
                                             mybir.ActivationFunctionType.Relu,
                                             bias=b1[:, 0:1])
                        p2 = qpool.tile([64, N // 2], FP, space="PSUM",
                                        tag="p2")
                        nc.tensor.matmul(out=p2, lhsT=w2t, rhs=h1,
                                         start=True, stop=True)
                        s2s = mpool.tile([64, N // 2], FP, tag="s2s")
                        nc.scalar.activation(s2s, p2,
                                             mybir.ActivationFunctionType.Copy)
                        # transpose [64, 128] chunks -> [128, 64] rows
                        ov = outt.rearrange("p (k f) -> p k f", f=64)
                        for k in range(N // 2 // 128):
                            pT = qpool.tile([128, 64], FP, space="PSUM",
                                            tag="pT")
                            nc.tensor.transpose(
                                out=pT, in_=s2s[:, k * 128:(k + 1) * 128],
                                identity=ident[0:64, 0:64])
                            nc.vector.tensor_tensor(
                                out=ov[:, 2 * k + h, :], in0=pT, in1=b2r,
                                op=ALU.add)
                    # rows of outt: col-block j = chunk 2k+h holds points
                    # n = 2*(128k + row) + h  (row = partition)
                    # DMA per col-block j: dst rows base + 2*128k + h + 2*row
                    od2 = out_d.rearrange("(a two) f -> a two f", two=2)
                    for h in range(2):
                        for k in range(N // 2 // 128):
                            j = 2 * k + h
                            st = blk * (N // 2) + 128 * k
                            nc.sync.dma_start(od2[st:st + 128, h, :],
                                              ov[:, j, :])
                    blk += 1

    nc.compile()
    return nc


# ---------------------------------------------------------------------------
# host-side data prep
# ---------------------------------------------------------------------------

def make_tables16(planes_list):
    """[3,16,150,W] fp32 -> per-scale fp16 delta table
    [16ch, (H-1)*3*(W-1)*4] with cell = ((iy*3)+pl)*(W-1)+ix,
    payload (v00, dy, dx, dxy)."""
    tabs = []
    for P in planes_list:
        v00 = P[:, :, :-1, :-1]
        v01 = P[:, :, :-1, 1:]
        v10 = P[:, :, 1:, :-1]
        v11 = P[:, :, 1:, 1:]
        dx = v01 - v00
        dy = v10 - v00
        dxy = v11 - v10 - v01 + v00
        t = np.stack([v00, dy, dx, dxy], axis=-1)      # [3,16,149,W-1,4]
        t = t.transpose(1, 2, 0, 3, 4)                 # [16,149,3,W-1,4]
        C = t.shape[0]
        tabs.append(np.ascontiguousarray(
            t.reshape(C, -1).astype(np.float16)))
    return tabs


def point_geometry(pts):
    """Host replica of the reference index math (float32 exact).
    Returns iy0c [N], wy [N], ix0c [N,4s,3p], wx [N,4s,3p]."""
    t = pts[:, 3].astype(np.float32)
    ay = np.float32(0.5 * (H - 1))
    iy = np.clip((t + 1.0) * ay, 0.0, np.float32(H - 1)).astype(np.float32)
    iy0 = np.floor(iy)
    iy0c = np.minimum(iy0, np.float32(H - 2)).astype(np.int32)
    wy = (iy - iy0c).astype(np.float32)

    ix0c = np.empty((pts.shape[0], NS, NP), np.int32)
    wx = np.empty((pts.shape[0], NS, NP), np.float32)
    for s in range(NS):
        W = WS[s]
        ax = np.float32(0.5 * (W - 1))
        for p in range(NP):
            x = pts[:, p].astype(np.float32)
            ix = np.clip((x + 1.0) * ax, 0.0, np.float32(W - 1)
                         ).astype(np.float32)
            i0 = np.floor(ix)
            i0c = np.minimum(i0, np.float32(W - 2)).astype(np.int32)
            ix0c[:, s, p] = i0c
            wx[:, s, p] = ix - i0c
    return iy0c, wy, ix0c, wx


def build_core_inputs(shard, tiles_per_bucket):
    """Per-core: bucket-sort, pad to N-multiples, emit idx/wx/wy arrays
    plus the slot->original-point permutation."""
    npts = shard.shape[0]
    iy0c, wy, ix0c, wx = point_geometry(shard)
    bkt = np.minimum(iy0c // YB, NBKT - 1)

    nb = int(sum(tiles_per_bucket))
    idx_a = np.zeros((nb, 128, NSLOT // 16), np.int16)
    wx_a = np.zeros((nb, 8, NSLOT), np.float16)
    wy_a = np.zeros((nb, 8, N // 2), np.float16)
    perm = np.full(nb * N, -1, np.int64)

    order = np.argsort(bkt, kind="stable")
    pos = 0
    blk = 0
    for b in range(NBKT):
        sel = order[bkt[order] == b]
        lo, win = bucket_rows(b)
        ntile = int(tiles_per_bucket[b])
        cnt = len(sel)
        for tti in range(ntile):
            pts_t = sel[tti * N:(tti + 1) * N]
            m = len(pts_t)
            # local slot arrays for this tile
            idx_g = np.zeros((8, NSLOT), np.int16)   # per group lists
            wx_g = np.zeros((8, NSLOT), np.float16)
            wy_g = np.zeros((8, N // 2), np.float16)
            if m > 0:
                n = np.arange(m)
                hh = n % 2
                sg = n // 2
                iyl = iy0c[pts_t] - lo
                for s in range(NS):
                    cpr = cells_per_row(s)
                    for p in range(NP):
                        cell = (iyl * NP + p) * (WS[s] - 1) + ix0c[pts_t, s, p]
                        gidx = 4 * hh + s
                        slot = sg * NP + p
                        idx_g[gidx, slot] = cell.astype(np.int16)
                        wx_g[gidx, slot] = wx[pts_t, s, p].astype(np.float16)
                for s in range(NS):
                    wy_g[4 * hh + s, sg] = wy[pts_t].astype(np.float16)
            # wrap idx into 16 partitions: idx i -> partition 16g + i%16,
            # col i//16
            for gi in range(8):
                idx_a[blk, 16 * gi:16 * gi + 16] = \
                    idx_g[gi].reshape(NSLOT // 16, 16).T
            wx_a[blk] = wx_g
            wy_a[blk] = wy_g
            perm[blk * N:blk * N + m] = pts_t
            blk += 1
        pos += cnt
    return idx_a, wx_a, wy_a, perm


def host_inputs(pts, planes_list, w0, b0, w1, b1, w2, b2):
    tabs = make_tables16(planes_list)

    percore = (pts.shape[0] + NCORES - 1) // NCORES
    shards = [pts[c * percore:(c + 1) * percore] for c in range(NCORES)]

    # per-bucket tile counts: max over cores (same program everywhere)
    counts = np.zeros((NCORES, NBKT), np.int64)
    for c, sh in enumerate(shards):
        iy0c, _, _, _ = point_geometry(sh)
        bk = np.minimum(iy0c // YB, NBKT - 1)
        for b in range(NBKT):
            counts[c, b] = int((bk == b).sum())
    tiles_per_bucket = [int(math.ceil(counts[:, b].max() / N))
                        for b in range(NBKT)]

    sel8 = np.zeros((8, 128), np.float16)
    for gi in range(8):
        sel8[gi, 16 * gi:16 * gi + 16] = 1.0

    shared = {
        **{f"tab{s}": tabs[s] for s in range(NS)},
        "sel8": sel8,
        "w0arr": np.ascontiguousarray(
            np.concatenate([w0.T, w0.T], axis=0).astype(np.float16)),
        "w1t": np.ascontiguousarray(w1.T.astype(np.float16)),
        "w2t": np.ascontiguousarray(w2.T.astype(np.float16)),     # [128,64]
        "b0c": np.ascontiguousarray(b0.reshape(128, 1).astype(np.float32)),
        "b1c": np.ascontiguousarray(b1.reshape(128, 1).astype(np.float32)),
        "b2r": np.ascontiguousarray(
            np.broadcast_to(b2.reshape(1, 64), (128, 64)).astype(np.float32)),
    }
    in_maps, perms = [], []
    for c in range(NCORES):
        idx_a, wx_a, wy_a, perm = build_core_inputs(shards[c],
                                                    tiles_per_bucket)
        in_maps.append({**shared, "idx": idx_a, "wx": wx_a, "wy": wy_a})
        perms.append(perm)
    return in_maps, perms, tiles_per_bucket, percore


# ---------------------------------------------------------------------------
# entry point
# ---------------------------------------------------------------------------

_CACHE = {}


def kernel(pts, planes_s0, planes_s1, planes_s2, planes_s3,
           w0, b0, w1, b1, w2, b2, _want_trace=False):
    from concourse.bass_utils import run_bass_kernel_spmd

    pts = np.asarray(pts, np.float32)
    planes = [np.asarray(p, np.float32)
              for p in (planes_s0, planes_s1, planes_s2, planes_s3)]
    in_maps, perms, tiles_per_bucket, percore = host_inputs(
        pts, planes,
        np.asarray(w0, np.float32), np.asarray(b0, np.float32),
        np.asarray(w1, np.float32), np.asarray(b1, np.float32),
        np.asarray(w2, np.float32), np.asarray(b2, np.float32))

    import time as _t
    key = tuple(tiles_per_bucket)
    if key not in _CACHE:
        t0 = _t.time()
        print(f"[kernel] building program nb={sum(tiles_per_bucket)}",
              flush=True)
        _CACHE[key] = build_program(tiles_per_bucket, num_devices=NCORES)
        print(f"[kernel] build done {_t.time()-t0:.1f}s", flush=True)
    nc = _CACHE[key]

    t0 = _t.time()
    print("[kernel] launching on 8 cores", flush=True)
    r = run_bass_kernel_spmd(nc, in_maps, core_ids=list(range(NCORES)),
                             trace=_want_trace)
    print(f"[kernel] run done {_t.time()-t0:.1f}s", flush=True)
    n = pts.shape[0]
    full = np.empty((n, 64), np.float32)
    for c in range(NCORES):
        dev = np.asarray(r.results[c]["out"])
        perm = perms[c]
        valid = perm >= 0
        base = c * percore
        full[base + perm[valid]] = dev[valid]
    if _want_trace:
        return full, r
    return full


if __name__ == "__main__":
    nc = build_program([1] * NBKT)
    print("built ok")



# revision 8
# speedup vs baseline: 1.8798x; 1.8798x over previous
"""Trainium2 Bass kernel: multiscale K-Planes lookup + MLP (v5).

Design (vs the ap_gather v2 baseline, which is Q7-request-bound at
~27 ns/idx => ~42 us/tile => ~13.7 ms total):
  - All 12 (scale,plane) bilinear lookups per point go through
    gpsimd.dma_gather (SWDGE): one 256B-row descriptor per lookup,
    12 calls x 1024 idx per 1024-point tile, rotating 4 SWDGE queues.
    Descriptors stripe across all 16 DMA engines; measured ~2 ns/desc
    effective => ~25 us/tile gather, fully off the Q7/DVE/PE path.
  - Tables live in DRAM as 256B rows [16c x (v00,dy,dx,dxy) f16 | pad],
    row id = iy0*2868 + scale_off[s] + pl*(W-1) + ix0 (int16 idx are
    window-relative; points are sorted by iy0 so a tile spans <=4 rows).
  - Gather output layout [128 = point, 96 = (blk,s,pl), 128B row]:
    per-lookup weights (1, wy, wx, wx*wy) are a tiny [128, 96, 4] f16
    tile multiplied with a stride-0 channel-broadcast AP - the DVE
    2x_1P mode only checks the innermost AP dim, so this runs at 2x.
  - val = sum_j w_j * g_j via two pair-collapse adds; plane product on
    dense 16-channel runs; feats PE-transposed to [64=(s,c), points]
    for the 3-layer MLP; output transposed back and stored f16.
"""

import numpy as np
from contextlib import ExitStack

import concourse.bass as bass
import concourse.bacc as bacc
import concourse.mybir as mybir
import concourse.tile as tile
from concourse import library_config

FP = mybir.dt.float32
F16 = mybir.dt.float16
I16 = mybir.dt.int16
ALU = mybir.AluOpType

H = 150
WS = (64, 128, 256, 512)
NP = 3
NS = 4
NCORES = 8
N = 1024                        # points per tile
NBLK = N // 128                 # 8
NCHUNK = NBLK * NS * NP         # 96 rows gathered per point-partition
NIDX = N * NS * NP              # 12288 descriptors per tile
ELEM = 128                      # row = 256B = [16c x 4j f16 | 64 pad]
CPR = NP * sum(w - 1 for w in WS)   # 2868 rows per iy
RROWS = (H - 1) * CPR           # 427332 table rows
WINROWS = 4 * CPR               # int16-relative row window per tile
GQ = 4                          # SWDGE queues
GCALLS = NIDX // 1024           # 12 gather calls per tile

SCALE_OFF = []
_off = 0
for _s in range(NS):
    SCALE_OFF.append(_off)
    _off += NP * (WS[_s] - 1)


# ---------------------------------------------------------------------------
# device program
# ---------------------------------------------------------------------------

def build_program(nb, bases, num_devices=1):
    """nb tiles; bases[t] = table window base row for tile t."""
    nc = bacc.Bacc("TRN2", target_bir_lowering=False, debug=False,
                   enable_asserts=False, num_devices=num_devices,
                   num_swdge_queues=GQ)

    tab_d = nc.dram_tensor("tab", [RROWS, ELEM], F16,
                           kind="ExternalInput").ap()
    idx_d = nc.dram_tensor("idx", [nb, 128, NIDX // 16], I16,
                           kind="ExternalInput").ap()
    w_d = nc.dram_tensor("wq", [nb, 128, NCHUNK * 4], F16,
                         kind="ExternalInput").ap()
    w0_d = nc.dram_tensor("w0t", [64, 128], F16, kind="ExternalInput").ap()
    w1_d = nc.dram_tensor("w1t", [128, 128], F16, kind="ExternalInput").ap()
    w2_d = nc.dram_tensor("w2t", [128, 64], F16, kind="ExternalInput").ap()
    b0_d = nc.dram_tensor("b0c", [128, 1], FP, kind="ExternalInput").ap()
    b1_d = nc.dram_tensor("b1c", [128, 1], FP, kind="ExternalInput").ap()
    b2_d = nc.dram_tensor("b2c", [64, 1], FP, kind="ExternalInput").ap()
    out_d = nc.dram_tensor("out", [nb * N, 64], F16,
                           kind="ExternalOutput").ap()

    from concourse.masks import make_identity

    with tile.TileContext(nc) as tc:
        with ExitStack() as ctx:
            cpool = ctx.enter_context(tc.tile_pool(name="cpool", bufs=1))
            ipool = ctx.enter_context(tc.tile_pool(name="ipool", bufs=2))
            gpool = ctx.enter_context(tc.tile_pool(name="gpool", bufs=2))
            vpool = ctx.enter_context(tc.tile_pool(name="vpool", bufs=2))
            mpool = ctx.enter_context(tc.tile_pool(name="mpool", bufs=2))
            qpool = ctx.enter_context(tc.tile_pool(name="qpool", bufs=1,
                                                   space="PSUM"))

            nc.gpsimd.load_library(library_config.mlp)

            ident = cpool.tile([128, 128], FP)
            make_identity(nc, ident)
            w0t = cpool.tile([64, 128], F16)
            nc.sync.dma_start(w0t, w0_d)
            w1t = cpool.tile([128, 128], F16)
            nc.sync.dma_start(w1t, w1_d)
            w2t = cpool.tile([128, 64], F16)
            nc.sync.dma_start(w2t, w2_d)
            b0 = cpool.tile([128, 1], FP)
            nc.sync.dma_start(b0, b0_d)
            b1 = cpool.tile([128, 1], FP)
            nc.sync.dma_start(b1, b1_d)
            b2 = cpool.tile([64, 1], FP)
            nc.sync.dma_start(b2, b2_d)

            for t in range(nb):
                idxt = ipool.tile([128, NIDX // 16], I16, tag="idx")
                nc.sync.dma_start(idxt, idx_d[t])
                wt = ipool.tile([128, NCHUNK * 4], F16, tag="w")
                nc.sync.dma_start(wt, w_d[t])

                # ---- gather: 12 x 1024 descriptors, rotating queues ----
                g = gpool.tile([128, NCHUNK * ELEM], F16, tag="g")
                gv = g.rearrange("p (q e) -> p q e", e=ELEM)
                base = int(bases[t])
                win = tab_d[base:base + WINROWS]
                for k in range(GCALLS):
                    nc.gpsimd.dma_gather(
                        gv[:, 8 * k:8 * (k + 1), :],
                        win,
                        idxt[:, 64 * k:64 * (k + 1)],
                        1024, 1024, ELEM, queue_num=k % GQ)

                # ---- interp: val = g . (1, wy, wx, wxy) ----
                g4 = g.rearrange("p (q s c j) -> p q s c j",
                                 s=2, c=16, j=4)[:, :, 0, :, :]
                P = vpool.tile([128, NCHUNK * 64], F16, tag="P")
                P4 = P.rearrange("p (q c j) -> p q c j", c=16, j=4)
                w4 = wt.rearrange("p (q j) -> p q j", j=4)
                wb = w4.unsqueeze(2).to_broadcast([128, NCHUNK, 16, 4])
                nc.vector.tensor_tensor(out=P4, in0=g4, in1=wb, op=ALU.mult)
                s2 = vpool.tile([128, NCHUNK * 32], F16, tag="s2")
                s2v = s2.rearrange("p (q c d) -> p q c d", c=16, d=2)
                nc.vector.tensor_tensor(out=s2v, in0=P4[:, :, :, 0:2],
                                        in1=P4[:, :, :, 2:4], op=ALU.add)
                val = vpool.tile([128, NCHUNK * 16], F16, tag="val")
                vv = val.rearrange("p (b s r c) -> p b s r c",
                                   b=NBLK, s=NS, r=NP)
                vvf = val.rearrange("p (q c) -> p q c", c=16)
                nc.vector.tensor_tensor(out=vvf, in0=s2v[:, :, :, 0],
                                        in1=s2v[:, :, :, 1], op=ALU.add)

                # ---- plane product -> feats [128, (blk, s, c)] ----
                pp = vpool.tile([128, NBLK * NS * 16], F16, tag="pp")
                ppv = pp.rearrange("p (b s c) -> p b s c", b=NBLK, s=NS)
                nc.vector.tensor_tensor(out=ppv, in0=vv[:, :, :, 0, :],
                                        in1=vv[:, :, :, 1, :], op=ALU.mult)
                feats = vpool.tile([128, NBLK * NS * 16], FP, tag="ft")
                fv = feats.rearrange("p (b s c) -> p b s c", b=NBLK, s=NS)
                nc.vector.tensor_tensor(out=fv, in0=ppv,
                                        in1=vv[:, :, :, 2, :], op=ALU.mult)

                # ---- transpose feats -> rhs [64 = (s,c), 1024 pts] ----
                # PSUM tags: A = fq/p1, B = p0/p2, C = po (5 banks, bufs=1)
                fq = qpool.tile([128, 1024], FP, tag="A")
                fb = feats.rearrange("p (b f) -> p b f", b=NBLK)
                for blk in range(NBLK):
                    nc.tensor.transpose(out=fq[0:64, 128 * blk:128 * (blk + 1)],
                                        in_=fb[:, blk, :],
                                        identity=ident)
                rhs = mpool.tile([64, 1024], F16, tag="rhs")
                nc.scalar.activation(rhs, fq[0:64, :],
                                     mybir.ActivationFunctionType.Copy)

                # ---- MLP ----
                p0 = qpool.tile([128, 1024], FP, tag="B")
                nc.tensor.matmul(out=p0[:, 0:512], lhsT=w0t,
                                 rhs=rhs[:, 0:512], start=True, stop=True)
                nc.tensor.matmul(out=p0[:, 512:1024], lhsT=w0t,
                                 rhs=rhs[:, 512:1024], start=True, stop=True)
                h0 = mpool.tile([128, 1024], F16, tag="h0")
                nc.scalar.activation(h0, p0,
                                     mybir.ActivationFunctionType.Relu,
                                     bias=b0[:, 0:1])
                p1 = qpool.tile([128, 1024], FP, tag="A")
                nc.tensor.matmul(out=p1[:, 0:512], lhsT=w1t,
                                 rhs=h0[:, 0:512], start=True, stop=True)
                nc.tensor.matmul(out=p1[:, 512:1024], lhsT=w1t,
                                 rhs=h0[:, 512:1024], start=True, stop=True)
                h1 = mpool.tile([128, 1024], F16, tag="h1")
                nc.scalar.activation(h1, p1,
                                     mybir.ActivationFunctionType.Relu,
                                     bias=b1[:, 0:1])
                p2 = qpool.tile([128, 1024], FP, tag="B")
                nc.tensor.matmul(out=p2[0:64, 0:512], lhsT=w2t,
                                 rhs=h1[:, 0:512], start=True, stop=True)
                nc.tensor.matmul(out=p2[0:64, 512:1024], lhsT=w2t,
                                 rhs=h1[:, 512:1024], start=True, stop=True)
                s16 = mpool.tile([64, 1024], FP, tag="s16")
                nc.scalar.activation(s16, p2[0:64, :],
                                     mybir.ActivationFunctionType.Identity,
                                     bias=b2[:, 0:1])

                # ---- transpose back -> [128 pts, (blk, c)] and store ----
                po = qpool.tile([128, 512], FP, tag="C")
                sv = s16.rearrange("p (b f) -> p b f", b=NBLK)
                for blk in range(NBLK):
                    nc.tensor.transpose(out=po[:, 64 * blk:64 * (blk + 1)],
                                        in_=sv[:, blk, :],
                                        identity=ident[0:64, 0:64])
                o16 = mpool.tile([128, 512], F16, tag="o16")
                nc.scalar.activation(o16, po,
                                     mybir.ActivationFunctionType.Copy)
                ov = out_d[t * N:(t + 1) * N].rearrange(
                    "(b p) c -> p b c", p=128)
                nc.sync.dma_start(ov, o16.rearrange("p (b c) -> p b c",
                                                    b=NBLK))

    nc.compile()
    return nc


# ---------------------------------------------------------------------------
# host-side data prep
# ---------------------------------------------------------------------------

def make_table(planes_list):
    """[3,16,150,W] fp32 per scale -> [RROWS, 128] f16 rows:
    row iy*CPR + SCALE_OFF[s] + pl*(W-1) + ix = [16c x (v00,dy,dx,dxy)|pad].
    """
    tab = np.zeros((H - 1, CPR, ELEM), np.float16)
    for s, P in enumerate(planes_list):
        v00 = P[:, :, :-1, :-1]
        v01 = P[:, :, :-1, 1:]
        v10 = P[:, :, 1:, :-1]
        v11 = P[:, :, 1:, 1:]
        dx = v01 - v00
        dy = v10 - v00
        dxy = v11 - v10 - v01 + v00
        t = np.stack([v00, dy, dx, dxy], axis=-1)    # [3,16,149,W-1,4]
        t = t.transpose(2, 0, 3, 1, 4)               # [149,3,W-1,16,4]
        W = P.shape[3]
        t = t.reshape(H - 1, NP * (W - 1), 64)
        tab[:, SCALE_OFF[s]:SCALE_OFF[s] + NP * (W - 1), 0:64] = t
    return np.ascontiguousarray(tab.reshape(RROWS, ELEM))


def point_geometry(pts):
    """iy0c [n], wy [n], ix0c [n,4s,3p], wx [n,4s,3p] (float32 exact)."""
    t = pts[:, 3].astype(np.float32)
    ay = np.float32(0.5 * (H - 1))
    iy = np.clip((t + 1.0) * ay, 0.0, np.float32(H - 1)).astype(np.float32)
    iy0 = np.floor(iy)
    iy0c = np.minimum(iy0, np.float32(H - 2)).astype(np.int32)
    wy = (iy - iy0c).astype(np.float32)
    n = pts.shape[0]
    ix0c = np.empty((n, NS, NP), np.int32)
    wx = np.empty((n, NS, NP), np.float32)
    for s in range(NS):
        W = WS[s]
        ax = np.float32(0.5 * (W - 1))
        for p in range(NP):
            x = pts[:, p].astype(np.float32)
            ix = np.clip((x + 1.0) * ax, 0.0, np.float32(W - 1)
                         ).astype(np.float32)
            i0 = np.floor(ix)
            i0c = np.minimum(i0, np.float32(W - 2)).astype(np.int32)
            ix0c[:, s, p] = i0c
            wx[:, s, p] = ix - i0c
    return iy0c, wy, ix0c, wx


def _sorted_geometry(shard):
    iy0c, wy, ix0c, wx = point_geometry(shard)
    order = np.argsort(iy0c, kind="stable")
    grow = (iy0c[order][:, None, None].astype(np.int64) * CPR
            + np.array(SCALE_OFF, np.int64)[None, :, None]
            + np.arange(NP)[None, None, :]
            * (np.array(WS, np.int64)[None, :, None] - 1)
            + ix0c[order])                            # [npts, 4, 3]
    return iy0c[order], wy[order], wx[order], grow, order


def build_core_inputs(geo, bases):
    """geo from _sorted_geometry; emit per-tile idx/weights arrays."""
    iy_s, wy_s, wx_s, grow, order = geo
    npts = iy_s.shape[0]
    nb = (npts + N - 1) // N
    idx_a = np.zeros((nb, 128, NIDX // 16), np.int16)
    w_a = np.zeros((nb, 128, NCHUNK * 4), np.float16)
    for t in range(nb):
        lo = t * N
        m = min(N, npts - lo)
        base = int(bases[t])
        rel = grow[lo:lo + m] - base
        assert rel.min() >= 0 and rel.max() < WINROWS, (
            t, rel.min(), rel.max())
        # flat idx position i = ((blk*NS + s)*NP + pl)*128 + p for point
        # n = blk*128 + p; wrap: position i -> (partition i%16, col i//16)
        nloc = np.arange(m)
        blk = nloc // 128
        p = nloc % 128
        i = ((blk[:, None, None] * NS
              + np.arange(NS)[None, :, None]) * NP
             + np.arange(NP)[None, None, :]) * 128 + p[:, None, None]
        flat_idx = np.zeros(NIDX, np.int16)
        flat_idx[i.reshape(-1)] = rel.astype(np.int16).reshape(-1)
        band = flat_idx.reshape(NIDX // 16, 16).T     # [16, 768]
        idx_a[t] = np.tile(band, (8, 1))
        # weights [128, (blk, s, pl), 4] = (1, wy, wx, wx*wy)
        wq = np.zeros((128, NBLK, NS, NP, 4), np.float32)
        wq[p, blk, :, :, 0] = 1.0
        wq[p, blk, :, :, 1] = wy_s[lo:lo + m, None, None]
        wq[p, blk, :, :, 2] = wx_s[lo:lo + m]
        wq[p, blk, :, :, 3] = (wx_s[lo:lo + m]
                               * wy_s[lo:lo + m, None, None])
        w_a[t] = wq.reshape(128, NCHUNK * 4).astype(np.float16)
    return idx_a, w_a


def host_inputs(pts, planes_list, w0, b0, w1, b1, w2, b2):
    tab = make_table(planes_list)
    percore = (pts.shape[0] + NCORES - 1) // NCORES
    shared = {
        "tab": tab,
        "w0t": np.ascontiguousarray(w0.T.astype(np.float16)),
        "w1t": np.ascontiguousarray(w1.T.astype(np.float16)),
        "w2t": np.ascontiguousarray(w2.T.astype(np.float16)),
        "b0c": np.ascontiguousarray(b0.reshape(128, 1).astype(np.float32)),
        "b1c": np.ascontiguousarray(b1.reshape(128, 1).astype(np.float32)),
        "b2c": np.ascontiguousarray(b2.reshape(64, 1).astype(np.float32)),
    }
    geos = []
    nb = 0
    for c in range(NCORES):
        geo = _sorted_geometry(pts[c * percore:(c + 1) * percore])
        geos.append(geo)
        nb = max(nb, (geo[0].shape[0] + N - 1) // N)
    # common per-tile window base = min over cores (program is shared)
    bases = np.full(nb, RROWS - WINROWS, np.int64)
    for geo in geos:
        iy_s = geo[0]
        for t in range(nb):
            lo = min(t * N, iy_s.shape[0] - 1)
            b = min(int(iy_s[lo]) * CPR, RROWS - WINROWS)
            bases[t] = min(bases[t], b)
    in_maps, orders = [], []
    for c in range(NCORES):
        idx_a, w_a = build_core_inputs(geos[c], bases)
        in_maps.append({**shared, "idx": idx_a, "wq": w_a})
        orders.append(geos[c][4])
    return in_maps, orders, bases, percore


# ---------------------------------------------------------------------------
# entry point
# ---------------------------------------------------------------------------

_CACHE = {}


def kernel(pts, planes_s0, planes_s1, planes_s2, planes_s3,
           w0, b0, w1, b1, w2, b2, _want_trace=False):
    from concourse.bass_utils import run_bass_kernel_spmd

    pts = np.asarray(pts, np.float32)
    planes = [np.asarray(p, np.float32)
              for p in (planes_s0, planes_s1, planes_s2, planes_s3)]
    in_maps, orders, bases, percore = host_inputs(
        pts, planes,
        np.asarray(w0, np.float32), np.asarray(b0, np.float32),
        np.asarray(w1, np.float32), np.asarray(b1, np.float32),
        np.asarray(w2, np.float32), np.asarray(b2, np.float32))

    import time as _t
    nb = len(bases)
    key = tuple(bases)
    if key not in _CACHE:
        t0 = _t.time()
        print(f"[kernel] building program nb={nb}", flush=True)
        _CACHE.clear()
        _CACHE[key] = build_program(nb, bases, num_devices=NCORES)
        print(f"[kernel] build done {_t.time()-t0:.1f}s", flush=True)
    nc = _CACHE[key]

    t0 = _t.time()
    print("[kernel] launching on 8 cores", flush=True)
    r = run_bass_kernel_spmd(nc, in_maps, core_ids=list(range(NCORES)),
                             trace=_want_trace)
    print(f"[kernel] run done {_t.time()-t0:.1f}s", flush=True)
    n = pts.shape[0]
    full = np.empty((n, 64), np.float32)
    for c in range(NCORES):
        dev = np.asarray(r.results[c]["out"]).astype(np.float32)
        order = orders[c]
        base = c * percore
        m = order.shape[0]
        full[base + order] = dev[:m]
    if _want_trace:
        return full, r
    return full


if __name__ == "__main__":
    nc = build_program(2, [0, 0])
    print("built ok")
